# revision 1
# baseline (speedup 1.0000x reference)
"""Trainium2 Bass kernel for nn_DeepLSTM: 2-layer LSTM (B=4096, T=1024, I=2, H=16, O=5).

Strategy (pure data parallel over batch, 8 cores x 512 batch each):
  - All recurrent compute per core, fully unrolled over T with a lag-pipeline:
    at tick t: layer-1 cell processes step t, layer-2 cell processes step t-2,
    the output linear processes step t-4.  The lag makes the three matmul
    stages independent within a tick and gives slack for SBUF->SBUF DMA dups.
  - Matmuls are "z-stationary": lhsT = z (feature-major bf16 activations
    [1;x;h...]), rhs = weight matrix (bf16), out = gates batch-major in PSUM
    [128 batch, 64 gates] per 128-batch block.  Biases are folded in via a
    constant-one row baked into the x stream.
  - Elementwise (sigmoid/tanh on ACT, multiplies/adds on DVE) is batch-major
    [128, small-free] merged across both cells and all 4 batch blocks per
    instruction to amortize fixed instruction overheads.
  - h is transposed back to feature-major via PE transpose (bf16) and copied
    PSUM->SBUF into the z buffers; secondary copies of h go via SBUF->SBUF DMA
    (slack >= 2 ticks).
"""

import os
import sys

import numpy as np

sys.path.insert(0, "/opt/trn_rl_repo")
os.environ.setdefault("JAX_PLATFORMS", "")

import concourse.bass as bass
import concourse.bacc as bacc
import concourse.mybir as mybir
import concourse.tile as tile
from concourse.bass_utils import run_bass_kernel_spmd

import ml_dtypes

BF16 = mybir.dt.bfloat16
F32 = mybir.dt.float32
AF = mybir.ActivationFunctionType
MUL = mybir.AluOpType.mult
ADD = mybir.AluOpType.add

B, T, I, H, O = 4096, 1024, 2, 16, 5
NCORES = 8
BC = B // NCORES      # 512 batch per core
NBLK = BC // 128      # 4 blocks of 128 batch
G = 4 * H             # 64 gate pre-activations per cell
LAG2 = 2              # layer-2 lag (ticks)
LAGY = 4              # output-linear lag (ticks)
CHUNK = 16            # z-slot chunk (ticks per z chunk tile)
YCH = 64              # y staging slots per DMA-out chunk

# z row layouts (feature-major, bf16).  Engine-instruction APs must start at
# partition 0/32/64/96, so h-copy targets sit at quadrant-aligned rows:
#  zA (19 rows): [0:16 h1][16 one][17:19 x]             -> lhsT of MM1
#  zB (48 rows): [0:16 h1][16 one][17:19 x][19:32 zero-pad][32:48 h2] -> MM2
#  zC (51 rows): [32:48 h2][48 one][49:51 x]            -> lhsT of MM3 (rows 32:51)
# Gate column order in the weight matrices is [i, f, o, g] so that sigmoid
# covers columns 0:48 and tanh covers 48:64 contiguously.


def build_program(Tloc=T, n_y_pad=8, trace_sim=False):
    nc = bacc.Bacc()

    xs_d = nc.declare_dram_parameter("xs", [Tloc, 16, BC], BF16, isOutput=False)
    w1_d = nc.declare_dram_parameter("w1", [19, G], BF16, isOutput=False)
    w2_d = nc.declare_dram_parameter("w2", [48, G], BF16, isOutput=False)
    wl_d = nc.declare_dram_parameter("wl", [19, n_y_pad], BF16, isOutput=False)
    id_d = nc.declare_dram_parameter("ident", [128, 128], BF16, isOutput=False)
    # host-built full images of the first two chunks of each z buffer
    # (zero h/pad rows + ones/x rows) so each tile has exactly ONE producer
    za_im = nc.declare_dram_parameter("za_im", [2, 19, CHUNK * BC], BF16, isOutput=False)
    zb_im = nc.declare_dram_parameter("zb_im", [2, 48, CHUNK * BC], BF16, isOutput=False)
    zc_im = nc.declare_dram_parameter("zc_im", [2, 51, CHUNK * BC], BF16, isOutput=False)
    y_d = nc.declare_dram_parameter("y", [Tloc, NBLK, n_y_pad, 128], F32, isOutput=True)

    nticks = Tloc + LAGY
    nchunks = (nticks + CHUNK) // CHUNK + 1

    with tile.TileContext(nc, trace_sim=trace_sim) as tc:
        with (
            tc.tile_pool(name="const", bufs=1) as cpool,
            tc.tile_pool(name="state", bufs=1) as spool,
            tc.tile_pool(name="za", bufs=2) as za_pool,
            tc.tile_pool(name="zb", bufs=2) as zb_pool,
            tc.tile_pool(name="zc", bufs=2) as zc_pool,
            tc.tile_pool(name="scr", bufs=2) as scr,
            tc.tile_pool(name="ystage", bufs=2) as ypool,
            tc.tile_pool(name="gpsum", bufs=2, space="PSUM") as gp_pool,
            tc.tile_pool(name="hpsum", bufs=2, space="PSUM") as hp_pool,
            tc.tile_pool(name="ypsum", bufs=2, space="PSUM") as yp_pool,
        ):
            # ---- constants ----
            w1_s = cpool.tile([19, G], BF16, name="w1_s")
            w2_s = cpool.tile([48, G], BF16, name="w2_s")
            wl_s = cpool.tile([51, n_y_pad], BF16, name="wl_s")
            id_s = cpool.tile([128, 128], BF16, name="id_s")
            nc.sync.dma_start(w1_s[:], w1_d[:])
            nc.sync.dma_start(w2_s[:], w2_d[:])
            nc.sync.dma_start(wl_s[32:51, :], wl_d[:])
            nc.sync.dma_start(id_s[:], id_d[:])

            # ---- persistent state: [tg(16) | c(16)] per (cell, blk) ----
            st = spool.tile([128, 2 * NBLK * 32], F32, name="st")
            stc = spool.tile([128, 1], F32, name="stc")
            nc.vector.memset(st[:], 0.0)
            # ACT-side carrier for the memset's DVE semaphore
            nc.scalar.copy(stc[:], st[:, 0:1])
            st4 = st.rearrange("p (c b s) -> p c b s", c=2, b=NBLK)

            # ---- z chunk tiles (allocated up front; pools rotate 2 slots) ----
            zas, zbs, zcs = [], [], []
            for k in range(nchunks):
                zas.append(za_pool.tile([19, CHUNK * BC], BF16, name=f"za{k}", tag="za"))
                zbs.append(zb_pool.tile([48, CHUNK * BC], BF16, name=f"zb{k}", tag="zb"))
                zcs.append(zc_pool.tile([51, CHUNK * BC], BF16, name=f"zc{k}", tag="zc"))

            def z_carriers(k):
                """Dummy 1-column weight loads so the PE absorbs each chunk
                DMA's semaphore wait on a dedicated cheap instruction (the
                Ldweights ISA slot fits only one embedded wait)."""
                nc.tensor.ldweights(zas[k][0:19, 0:1])
                nc.tensor.ldweights(zbs[k][0:48, 0:1])
                nc.tensor.ldweights(zcs[k][32:51, 0:1], tile_position=(32, 0))

            def x_dma(k):
                """Fill chunk k's x/one rows from the host x-stream (k >= 2)."""
                t0 = k * CHUNK
                n = min(CHUNK, Tloc - t0)
                if n <= 0:
                    return
                src = xs_d[t0 : t0 + n]  # [n, 16, BC]
                view = lambda z, r0, nr: z[r0 : r0 + nr].rearrange(
                    "r (t bb) -> r t bb", t=CHUNK
                )[:, :n, :]
                # dest partition dim must be first: src [n,16,BC] -> AP [16, n, BC]
                srcT = src.rearrange("t r bb -> r t bb")
                nc.sync.dma_start(view(zas[k], 16, 3), srcT[0:3])
                # zB also gets its zero pad rows (19:32) refreshed by this DMA
                nc.sync.dma_start(view(zbs[k], 16, 16), srcT)
                nc.sync.dma_start(view(zcs[k], 48, 3), srcT[0:3])
                z_carriers(k)

            for k in (0, 1):
                nc.sync.dma_start(zas[k][:], za_im[k])
                nc.sync.dma_start(zbs[k][:], zb_im[k])
                nc.sync.dma_start(zcs[k][:], zc_im[k])
                z_carriers(k)
            # carriers for the weight/identity DMAs (moving operands)
            nc.tensor.ldweights(w1_s[:, 0:1])
            nc.tensor.ldweights(w2_s[:, 0:1])
            nc.tensor.ldweights(wl_s[32:51, 0:1], tile_position=(32, 0))
            nc.tensor.ldweights(id_s[:, 0:1])

            ystages = []
            nych = (Tloc + YCH - 1) // YCH
            for k in range(nych):
                ystages.append(
                    ypool.tile([128, YCH * NBLK * n_y_pad], F32, name=f"ys{k}", tag="ys")
                )

            def slot(t):
                return (zas[t // CHUNK], zbs[t // CHUNK], zcs[t // CHUNK], t % CHUNK)

            yp_tiles = {}

            for tk in range(nticks):
                cell1 = tk < Tloc             # layer-1 step tk
                s2 = tk - LAG2                # layer-2 step
                cell2 = 0 <= s2 < Tloc
                s3 = tk - LAGY                # linear step
                celly = 0 <= s3 < Tloc

                if tk % CHUNK == 0 and tk // CHUNK + 2 < nchunks:
                    x_dma(tk // CHUNK + 2)

                # ---------- matmuls ----------
                gp = gp_pool.tile([128, 512], F32, name="gp", tag="gp")
                gp4 = gp.rearrange("p (c b g) -> p c b g", c=2, b=NBLK)
                if cell1:
                    za, _, _, s = slot(tk)
                    for b in range(NBLK):
                        nc.tensor.matmul(
                            gp[:, b * G : (b + 1) * G],
                            za[0:19, s * BC + b * 128 : s * BC + (b + 1) * 128],
                            w1_s[:],
                        )
                if cell2:
                    _, zb, _, s = slot(s2)
                    for b in range(NBLK):
                        nc.tensor.matmul(
                            gp[:, 256 + b * G : 256 + (b + 1) * G],
                            zb[0:48, s * BC + b * 128 : s * BC + (b + 1) * 128],
                            w2_s[:],
                        )
                if celly:
                    _, _, zc, s = slot(s3)
                    if s3 % 2 == 0:
                        yp_tiles[s3 // 2] = yp_pool.tile(
                            [128, 2 * n_y_pad * NBLK], F32, name="yp", tag="yp"
                        )
                    yp = yp_tiles[s3 // 2]
                    for b in range(NBLK):
                        nc.tensor.matmul(
                            yp[
                                :,
                                (s3 % 2) * n_y_pad * NBLK
                                + b * n_y_pad : (s3 % 2) * n_y_pad * NBLK
                                + (b + 1) * n_y_pad,
                            ],
                            zc[32:51, s * BC + b * 128 : s * BC + (b + 1) * 128],
                            wl_s[32:51, :],
                        )

                # ---------- elementwise (merged over active cells) ----------
                if cell1 or cell2:
                    c0 = 0 if cell1 else 1       # first active cell index
                    ncell = (1 if cell1 else 0) + (1 if cell2 else 0)
                    cs = slice(c0, c0 + ncell)

                    sig = scr.tile([128, 2 * NBLK * 48], F32, name="sig", tag="sig")
                    sg4 = sig.rearrange("p (c b s) -> p c b s", c=2, b=NBLK)
                    prod = scr.tile([128, 2 * NBLK * 32], F32, name="prod", tag="prod")
                    pr4 = prod.rearrange("p (c b s) -> p c b s", c=2, b=NBLK)
                    tcs = scr.tile([128, 2 * NBLK * 16], F32, name="tcs", tag="tcs")
                    tc4 = tcs.rearrange("p (c b s) -> p c b s", c=2, b=NBLK)
                    hbuf = scr.tile([128, NBLK * 32], BF16, name="hbuf", tag="hbuf")
                    # hbuf cols: blk*32 + cell*16 + h  (cell dim middle for transpose)
                    hb4 = hbuf.rearrange("p (b c s) -> p c b s", b=NBLK, c=2)

                    # sigmoid over [i,f,o] of the active cells
                    nc.scalar.activation(
                        sg4[:, cs], gp4[:, cs, :, 0:48], AF.Sigmoid
                    )
                    # tanh(g) -> st tg slot
                    nc.scalar.activation(
                        st4[:, cs, :, 0:16], gp4[:, cs, :, 48:64], AF.Tanh
                    )
                    # [i*tg | f*c]
                    nc.vector.tensor_mul(
                        pr4[:, cs], sg4[:, cs, :, 0:32], st4[:, cs]
                    )
                    # c' = i*tg + f*c   (in place into st c slot)
                    nc.vector.tensor_add(
                        st4[:, cs, :, 16:32],
                        pr4[:, cs, :, 0:16],
                        pr4[:, cs, :, 16:32],
                    )
                    # tanh(c')
                    nc.scalar.activation(
                        tc4[:, cs], st4[:, cs, :, 16:32], AF.Tanh
                    )
                    # h = sig_o * tanh(c')  (bf16, layout for PE transpose)
                    nc.vector.tensor_mul(
                        hb4[:, cs], sg4[:, cs, :, 32:48], tc4[:, cs]
                    )

                    # ---------- h transpose + distribution ----------
                    # hT rows 0:16 = h1(tk)^T, rows 32:48 = h2(s2)^T
                    # (quadrant-aligned so both halves are legally readable)
                    hT = hp_pool.tile([64, 512], BF16, name="hT", tag="hT")
                    if cell1:
                        for b in range(NBLK):
                            nc.tensor.transpose(
                                hT[0:16, b * 128 : (b + 1) * 128],
                                hbuf[:, b * 32 : b * 32 + 16],
                                id_s[:],
                            )
                        zan, _, _, sn = slot(tk + 1)
                        nc.vector.tensor_copy(
                            zan[0:16, sn * BC : (sn + 1) * BC], hT[0:16, :]
                        )
                        # dup h1(tk) into zB slot tk (read at tick tk+LAG2)
                        _, zbn, _, sb = slot(tk)
                        nc.vector.tensor_copy(
                            zbn[0:16, sb * BC : (sb + 1) * BC],
                            zan[0:16, sn * BC : (sn + 1) * BC],
                        )
                    if cell2:
                        for b in range(NBLK):
                            nc.tensor.transpose(
                                hT[32:48, b * 128 : (b + 1) * 128],
                                hbuf[:, b * 32 + 16 : b * 32 + 32],
                                id_s[:],
                            )
                        _, zbn, _, sb = slot(s2 + 1)
                        nc.scalar.copy(
                            zbn[32:48, sb * BC : (sb + 1) * BC], hT[32:48, :]
                        )
                        # dup h2(s2) into zC slot s2 (read at tick s2+LAGY)
                        _, _, zcn, sc = slot(s2)
                        nc.vector.tensor_copy(
                            zcn[32:48, sc * BC : (sc + 1) * BC],
                            zbn[32:48, sb * BC : (sb + 1) * BC],
                        )

                # ---------- y evacuation ----------
                if celly and (s3 % 2 == 1 or s3 == Tloc - 1):
                    yp = yp_tiles.pop(s3 // 2)
                    ys = ystages[s3 // YCH]
                    lo = (s3 % YCH) // 2 * 2  # first slot of this pair within chunk
                    nsl = (s3 % 2) + 1
                    nc.vector.tensor_copy(
                        ys[:, lo * n_y_pad * NBLK : (lo + nsl) * n_y_pad * NBLK],
                        yp[:, 0 : nsl * n_y_pad * NBLK],
                    )
                if celly and (s3 % YCH == YCH - 1 or s3 == Tloc - 1):
                    k = s3 // YCH
                    t0 = k * YCH
                    n = min(YCH, Tloc - t0)
                    ys = ystages[k]
                    src = ys.rearrange("p (t b o) -> p t b o", t=YCH, b=NBLK)[
                        :, 0:n
                    ]
                    dst = y_d[t0 : t0 + n].rearrange("t b o p -> p t b o")
                    nc.sync.dma_start(dst, src)

    return nc


_prog_cache = {}


def _get_program(Tloc):
    if Tloc not in _prog_cache:
        nc = build_program(Tloc)
        nc.finalize()
        _prog_cache[Tloc] = nc
    return _prog_cache[Tloc]


def _prep_inputs(x, W_ih1, W_hh1, b_ih1, b_hh1, W_ih2, W_hh2, b_ih2, b_hh2, W_l, b_l):
    bf = ml_dtypes.bfloat16
    Tloc = x.shape[1]
    perm = np.r_[0:16, 16:32, 48:64, 32:48]  # [i,f,g,o] -> [i,f,o,g]

    w1 = np.zeros((19, G), np.float32)
    w1[0:16] = W_hh1.T
    w1[16] = b_ih1 + b_hh1
    w1[17:19] = W_ih1.T
    w1 = w1[:, perm]

    w2 = np.zeros((48, G), np.float32)
    w2[0:16] = W_ih2.T[2:18]
    w2[16] = b_ih2 + b_hh2
    w2[17:19] = W_ih2.T[0:2]
    w2[32:48] = W_hh2.T
    w2 = w2[:, perm]

    wl = np.zeros((19, 8), np.float32)
    wl[0:16, :O] = W_l.T[2:18]
    wl[16, :O] = b_l
    wl[17:19, :O] = W_l.T[0:2]

    ident = np.eye(128, dtype=np.float32)

    maps = []
    for c in range(NCORES):
        xc = x[c * BC : (c + 1) * BC]  # [BC, T, I]
        xs = np.zeros((Tloc, 16, BC), np.float32)
        xs[:, 0] = 1.0
        xs[:, 1:3] = xc.transpose(1, 2, 0)
        xsb = xs.astype(bf)

        def img(rows, r0):
            im = np.zeros((2, rows, CHUNK * BC), bf)
            for k in range(2):
                n = min(CHUNK, Tloc - k * CHUNK)
                if n > 0:
                    blk = xsb[k * CHUNK : k * CHUNK + n, 0:3]  # [n, 3, BC]
                    im[k, r0 : r0 + 3, : n * BC] = blk.transpose(1, 0, 2).reshape(
                        3, n * BC
                    )
            return im

        maps.append(
            dict(
                xs=xsb,
                w1=w1.astype(bf),
                w2=w2.astype(bf),
                wl=wl.astype(bf),
                ident=ident.astype(bf),
                za_im=img(19, 16),
                zb_im=img(48, 16),
                zc_im=img(51, 48),
            )
        )
    return maps


def _assemble(results, Tloc):
    y = np.empty((B, Tloc, O), np.float32)
    for c in range(NCORES):
        yc = results[c]["y"]  # [T, NBLK, 8, 128]
        yc = yc[:, :, :O, :]  # [T, NBLK, O, 128]
        yc = np.transpose(yc, (1, 3, 0, 2)).reshape(BC, Tloc, O)
        y[c * BC : (c + 1) * BC] = yc
    return y


def run(inputs, trace=False, **kw):
    x = np.asarray(inputs["x"])
    Tloc = x.shape[1]
    nc = _get_program(Tloc)
    in_maps = _prep_inputs(**{k: np.asarray(v) for k, v in inputs.items()})
    res = run_bass_kernel_spmd(nc, in_maps, list(range(NCORES)), trace=trace, **kw)
    return _assemble(res.results, Tloc), res


def kernel(**inputs):
    y, _ = run(inputs)
    return y



# revision 2
# speedup vs baseline: 1.0547x; 1.0547x over previous
"""Trainium2 Bass kernel for nn_DeepLSTM: 2-layer LSTM (B=4096, T=1024, I=2, H=16, O=5).

V2 design (pure data parallel over batch, 8 cores x 512 batch each):
  - Lag pipeline: tick t computes cell1 step t, cell2 step t-1, linear step t-2.
    With lag-1 for cell2, both cells read the SAME h1(t-1) rows; with lag-2 for
    y, it reads the same h2(t-2) rows as cell2 -- so each hidden state has
    exactly ONE SBUF home and the per-tick h distribution is a single
    [32, 512] PSUM->SBUF copy.
  - One combined stationary per (tick, block): z = [h1(16) | h2(16) | 1 | x(t)
    | x(t-1) | x(t-2) | pad] (40 rows x 128 batch, bf16).  Two matmuls stream
    the static weight matrix columns over it: gates (N=128: [i f o g] x 2
    cells) and y (N=8).  4 LDW + 8 MM + 4 transposes per tick on PE.
  - Elementwise batch-major, merged across cells and blocks:
    sigmoid(ifo) + tanh(g) + mul + add + tanh(c) + mul = 6 ACT/DVE ops per
    tick; c-state kept in bf16 for DVE 2x modes.
  - x stream host-packed as [chunk][8 rows][16 ticks][512 batch] so each
    16-tick chunk is ONE dma_start with 8 fat fully-contiguous descriptors.
  - y staged 64 steps then written with [b][p][t][o]-major DRAM layout
    (2KB descriptor runs); final transpose done on host.
"""

import os
import sys

import numpy as np

sys.path.insert(0, "/opt/trn_rl_repo")
os.environ.setdefault("JAX_PLATFORMS", "")

import concourse.bass as bass
import concourse.bacc as bacc
import concourse.mybir as mybir
import concourse.tile as tile
from concourse.bass_utils import run_bass_kernel_spmd

import ml_dtypes

BF16 = mybir.dt.bfloat16
F32 = mybir.dt.float32
AF = mybir.ActivationFunctionType

B, T, I, H, O = 4096, 1024, 2, 16, 5
NCORES = 8
BC = B // NCORES      # 512 batch per core
NBLK = BC // 128      # 4 blocks of 128 batch
CHUNK = 16            # ticks per z chunk tile
YCH = 64              # y steps per staging tile / output DMA
KZ = 40               # z rows: h1 16 | h2 16 | 1 | x(t) 2 | x(t-1) 2 | x(t-2) 2 | pad
NPRE = 4              # chunks of x prefetch distance

# z row layout
R_H1, R_H2, R_ONE, R_XT, R_XT1, R_XT2, R_PAD = 0, 16, 32, 33, 35, 37, 39
# weight cols per block of gp: [i1 f1 o1 g1 | i2 f2 o2 g2]; y cols 128:136


def build_program(Tloc=T, trace_sim=False):
    nc = bacc.Bacc()
    nticks = Tloc + 2
    nch = (nticks + 1 + CHUNK - 1) // CHUNK  # cover copy at tick nticks-1 -> z(nticks)

    xq_d = nc.declare_dram_parameter("xq", [nch, 8, CHUNK * BC], BF16, isOutput=False)
    w_d = nc.declare_dram_parameter("w", [KZ, 136], BF16, isOutput=False)
    id_d = nc.declare_dram_parameter("ident", [128, 128], BF16, isOutput=False)
    y_d = nc.declare_dram_parameter("y", [NBLK, 128, Tloc, 8], F32, isOutput=True)

    with tile.TileContext(nc, trace_sim=trace_sim) as tc:
        with (
            tc.tile_pool(name="const", bufs=1) as cpool,
            tc.tile_pool(name="state", bufs=1) as spool,
            tc.tile_pool(name="z", bufs=6) as zpool,
            tc.tile_pool(name="scr", bufs=2) as scr,
            tc.tile_pool(name="ystage", bufs=2) as ypool,
            tc.tile_pool(name="gpsum", bufs=3, space="PSUM") as gp_pool,
            tc.tile_pool(name="hpsum", bufs=2, space="PSUM") as hp_pool,
        ):
            # ---- constants ----
            w_s = cpool.tile([KZ, 136], BF16, name="w_s")
            id_s = cpool.tile([128, 128], BF16, name="id_s")
            nc.sync.dma_start(w_s[:], w_d[:])
            nc.sync.dma_start(id_s[:], id_d[:])
            nc.tensor.ldweights(w_s[:, 0:1])
            nc.tensor.ldweights(id_s[:, 0:1])

            # ---- persistent state: [tg(16) | c(16)] per (blk, cell) ----
            st = spool.tile([128, NBLK * 2 * 32], BF16, name="st")
            stc = spool.tile([128, 1], F32, name="stc")
            nc.vector.memset(st[:], 0.0)
            nc.scalar.copy(stc[:], st[:, 0:1])  # ACT-side carrier for memset sem
            st4 = st.rearrange("p (b c s) -> p b c s", b=NBLK, c=2)

            # ---- z chunk tiles ----
            zs = [zpool.tile([KZ, CHUNK * BC], BF16, name=f"z{k}", tag="z") for k in range(nch)]
            # zero the h rows read before first writes (ticks 0 and 1)
            nc.vector.memset(zs[0][0:32, 0 : 2 * BC], 0.0)
            nc.tensor.ldweights(zs[0][0:32, 0:1])

            def x_dma(k):
                nc.sync.dma_start(zs[k][32:40, :], xq_d[k])
                nc.tensor.ldweights(zs[k][0:40, 0:1])

            for k in range(min(NPRE, nch)):
                x_dma(k)

            ystages = []
            nych = (Tloc + YCH - 1) // YCH
            for k in range(nych):
                ystages.append(
                    ypool.tile([128, YCH * NBLK * 8], F32, name=f"ys{k}", tag="ys")
                )

            for t in range(nticks):
                if t % CHUNK == 0 and t // CHUNK + NPRE < nch:
                    x_dma(t // CHUNK + NPRE)

                z = zs[t // CHUNK]
                s = t % CHUNK
                zcol = lambda b: z[0:KZ, s * BC + b * 128 : s * BC + (b + 1) * 128]

                # ---------- matmuls ----------
                # gp spans 2 PSUM banks; blocks at 256-col offsets so each
                # [*, 136] matmul output stays within one bank.
                gp = gp_pool.tile([128, 1024], F32, name="gp", tag="gp")
                gp4 = gp.rearrange("p (b c k) -> p b c k", b=NBLK, c=4)
                nw = 136 if t >= 2 else 128  # y cols only once valid
                for b in range(NBLK):
                    nc.tensor.matmul(
                        gp[:, b * 256 : b * 256 + nw], zcol(b), w_s[:, 0:nw]
                    )

                # ---------- elementwise ----------
                cs = slice(0, 1) if t == 0 else slice(0, 2)

                sg = scr.tile([128, NBLK * 2 * 48], BF16, name="sg", tag="sg")
                sg4 = sg.rearrange("p (b c s) -> p b c s", b=NBLK, c=2)
                prod = scr.tile([128, NBLK * 2 * 32], BF16, name="prod", tag="prod")
                pr4 = prod.rearrange("p (b c s) -> p b c s", b=NBLK, c=2)
                tcs = scr.tile([128, NBLK * 2 * 16], BF16, name="tcs", tag="tcs")
                tc4 = tcs.rearrange("p (b c s) -> p b c s", b=NBLK, c=2)
                hbuf = scr.tile([128, NBLK * 32], BF16, name="hbuf", tag="hbuf")
                hb4 = hbuf.rearrange("p (b c s) -> p b c s", b=NBLK, c=2)

                # sigmoid over [i, f] of active cells (spine)
                nc.scalar.activation(sg4[:, :, cs, 0:32], gp4[:, :, cs, 0:32], AF.Sigmoid)
                # tanh(g) -> st tg slot (spine)
                nc.scalar.activation(st4[:, :, cs, 0:16], gp4[:, :, cs, 48:64], AF.Tanh)
                # [i*tg | f*c]
                nc.vector.tensor_mul(pr4[:, :, cs], sg4[:, :, cs, 0:32], st4[:, :, cs])
                # sigmoid(o) off-spine: overlaps the DVE mul/add
                nc.scalar.activation(sg4[:, :, cs, 32:48], gp4[:, :, cs, 32:48], AF.Sigmoid)
                # c' = i*tg + f*c  (into st c slot)
                nc.vector.tensor_add(
                    st4[:, :, cs, 16:32], pr4[:, :, cs, 0:16], pr4[:, :, cs, 16:32]
                )
                # tanh(c')
                nc.scalar.activation(tc4[:, :, cs], st4[:, :, cs, 16:32], AF.Tanh)
                # h = sig_o * tanh(c')
                nc.vector.tensor_mul(hb4[:, :, cs], sg4[:, :, cs, 32:48], tc4[:, :, cs])

                # ---------- h transpose + single copy ----------
                hT = hp_pool.tile([32, 512], BF16, name="hT", tag="hT")
                ncl = 16 if t == 0 else 32
                for b in range(NBLK):
                    nc.tensor.transpose(
                        hT[0:ncl, b * 128 : (b + 1) * 128],
                        hbuf[:, b * 32 : b * 32 + ncl],
                        id_s[:],
                    )
                zn = zs[(t + 1) // CHUNK]
                sn = (t + 1) % CHUNK
                nc.vector.tensor_copy(
                    zn[0:ncl, sn * BC : (sn + 1) * BC], hT[0:ncl, :]
                )

                # ---------- y evacuation ----------
                if t >= 2:
                    s3 = t - 2
                    ys = ystages[s3 // YCH]
                    ys4 = ys.rearrange("p (b ts o) -> p b ts o", b=NBLK, ts=YCH)
                    pos = s3 % YCH
                    nc.vector.tensor_copy(
                        ys4[:, :, pos : pos + 1, :], gp4[:, :, 2:3, 0:8]
                    )
                    if pos == YCH - 1 or s3 == Tloc - 1:
                        k = s3 // YCH
                        t0 = k * YCH
                        n = min(YCH, Tloc - t0)
                        src = ystages[k].rearrange(
                            "p (b ts o) -> p b ts o", b=NBLK, ts=YCH
                        )[:, :, 0:n, :]
                        dst = y_d[:, :, t0 : t0 + n, :].rearrange("b p ts o -> p b ts o")
                        nc.sync.dma_start(dst, src)

    return nc


_prog_cache = {}


def _get_program(Tloc):
    if Tloc not in _prog_cache:
        nc = build_program(Tloc)
        nc.finalize()
        _prog_cache[Tloc] = nc
    return _prog_cache[Tloc]


def _prep_weights(W_ih1, W_hh1, b_ih1, b_hh1, W_ih2, W_hh2, b_ih2, b_hh2, W_l, b_l):
    """Build the combined [KZ, 136] weight/bias matrix (f32; cast later)."""
    w = np.zeros((KZ, 136), np.float32)
    # torch gate order rows: [i, f, g, o] x16 ; our col order per cell: [i f o g]
    perm = np.r_[0:16, 16:32, 48:64, 32:48]  # -> [i, f, o, g]

    def cell_cols(c):
        return slice(c * 64, (c + 1) * 64)

    # cell 1 (cols 0:64)
    w[R_H1:R_H1 + 16, 0:64] = W_hh1.T[:, perm]
    w[R_ONE, 0:64] = (b_ih1 + b_hh1)[perm]
    w[R_XT:R_XT + 2, 0:64] = W_ih1.T[:, perm]
    # cell 2 (cols 64:128); x2 = [x, h1]
    w[R_H1:R_H1 + 16, 64:128] = W_ih2.T[2:18][:, perm]
    w[R_H2:R_H2 + 16, 64:128] = W_hh2.T[:, perm]
    w[R_ONE, 64:128] = (b_ih2 + b_hh2)[perm]
    w[R_XT1:R_XT1 + 2, 64:128] = W_ih2.T[0:2][:, perm]
    # y (cols 128:136); x3 = [x, h2]
    w[R_H2:R_H2 + 16, 128:133] = W_l.T[2:18]
    w[R_ONE, 128:133] = b_l
    w[R_XT2:R_XT2 + 2, 128:133] = W_l.T[0:2]
    return w


def _prep_inputs(x, W_ih1, W_hh1, b_ih1, b_hh1, W_ih2, W_hh2, b_ih2, b_hh2, W_l, b_l):
    bf = ml_dtypes.bfloat16
    Tloc = x.shape[1]
    nticks = Tloc + 2
    nch = (nticks + 1 + CHUNK - 1) // CHUNK
    w = _prep_weights(
        W_ih1, W_hh1, b_ih1, b_hh1, W_ih2, W_hh2, b_ih2, b_hh2, W_l, b_l
    ).astype(bf)
    ident = np.eye(128, dtype=np.float32).astype(bf)

    maps = []
    for c in range(NCORES):
        xc = np.ascontiguousarray(x[c * BC : (c + 1) * BC])  # [BC, T, 2]
        xt = np.zeros((nch * CHUNK, 2, BC), np.float32)
        xt[:Tloc] = xc.transpose(1, 2, 0)
        # xq rows: [ones | x(t) | x(t-1) | x(t-2) | zero]
        xq = np.zeros((nch * CHUNK, 8, BC), np.float32)
        xq[:, 0] = 1.0
        xq[:, 1:3] = xt
        xq[1:, 3:5] = xt[:-1]
        xq[2:, 5:7] = xt[:-2]
        xqb = (
            xq.astype(bf)
            .reshape(nch, CHUNK, 8, BC)
            .transpose(0, 2, 1, 3)  # [nch, 8, CHUNK, BC]
            .reshape(nch, 8, CHUNK * BC)
        )
        maps.append(dict(xq=np.ascontiguousarray(xqb), w=w, ident=ident))
    return maps


def _assemble(results, Tloc):
    y = np.empty((B, Tloc, O), np.float32)
    for c in range(NCORES):
        yc = results[c]["y"]  # [NBLK, 128, Tloc, 8]
        y[c * BC : (c + 1) * BC] = yc.reshape(BC, Tloc, 8)[:, :, :O]
    return y


def run(inputs, trace=False, **kw):
    x = np.asarray(inputs["x"])
    Tloc = x.shape[1]
    nc = _get_program(Tloc)
    in_maps = _prep_inputs(**{k: np.asarray(v) for k, v in inputs.items()})
    res = run_bass_kernel_spmd(nc, in_maps, list(range(NCORES)), trace=trace, **kw)
    return _assemble(res.results, Tloc), res


def kernel(**inputs):
    y, _ = run(inputs)
    return y


# revision 3
# speedup vs baseline: 1.0823x; 1.0262x over previous
"""Trainium2 Bass kernel for nn_DeepLSTM: 2-layer LSTM (B=4096, T=1024, I=2, H=16, O=5).

V2 design (pure data parallel over batch, 8 cores x 512 batch each):
  - Lag pipeline: tick t computes cell1 step t, cell2 step t-1, linear step t-2.
    With lag-1 for cell2, both cells read the SAME h1(t-1) rows; with lag-2 for
    y, it reads the same h2(t-2) rows as cell2 -- so each hidden state has
    exactly ONE SBUF home and the per-tick h distribution is a single
    [32, 512] PSUM->SBUF copy.
  - One combined stationary per (tick, block): z = [h1(16) | h2(16) | 1 | x(t)
    | x(t-1) | x(t-2) | pad] (40 rows x 128 batch, bf16).  Two matmuls stream
    the static weight matrix columns over it: gates (N=128: [i f o g] x 2
    cells) and y (N=8).  4 LDW + 8 MM + 4 transposes per tick on PE.
  - Elementwise batch-major, merged across cells and blocks:
    sigmoid(ifo) + tanh(g) + mul + add + tanh(c) + mul = 6 ACT/DVE ops per
    tick; c-state kept in bf16 for DVE 2x modes.
  - x stream host-packed as [chunk][8 rows][16 ticks][512 batch] so each
    16-tick chunk is ONE dma_start with 8 fat fully-contiguous descriptors.
  - y staged 64 steps then written with [b][p][t][o]-major DRAM layout
    (2KB descriptor runs); final transpose done on host.
"""

import os
import sys

import numpy as np

sys.path.insert(0, "/opt/trn_rl_repo")
os.environ.setdefault("JAX_PLATFORMS", "")

import concourse.bass as bass
import concourse.bacc as bacc
import concourse.mybir as mybir
import concourse.tile as tile
from concourse.bass_utils import run_bass_kernel_spmd

import ml_dtypes

BF16 = mybir.dt.bfloat16
F32 = mybir.dt.float32
AF = mybir.ActivationFunctionType

B, T, I, H, O = 4096, 1024, 2, 16, 5
NCORES = 8
BC = B // NCORES      # 512 batch per core
NBLK = BC // 128      # 4 blocks of 128 batch
CHUNK = 16            # ticks per z chunk tile
YCH = 64              # y steps per staging tile / output DMA
KZ = 40               # z rows: h1 16 | h2 16 | 1 | x(t) 2 | x(t-1) 2 | x(t-2) 2 | pad
NPRE = 4              # chunks of x prefetch distance

# z row layout
R_H1, R_H2, R_ONE, R_XT, R_XT1, R_XT2, R_PAD = 0, 16, 32, 33, 35, 37, 39
# weight cols per block of gp: [i1 f1 o1 g1 | i2 f2 o2 g2]; y cols 128:136


def build_program(Tloc=T, trace_sim=False):
    nc = bacc.Bacc()
    nticks = Tloc + 2
    nch = (nticks + 1 + CHUNK - 1) // CHUNK  # cover copy at tick nticks-1 -> z(nticks)

    xq_d = nc.declare_dram_parameter("xq", [nch, 8, CHUNK * BC], BF16, isOutput=False)
    w_d = nc.declare_dram_parameter("w", [KZ, 136], BF16, isOutput=False)
    id_d = nc.declare_dram_parameter("ident", [128, 128], BF16, isOutput=False)
    y_d = nc.declare_dram_parameter("y", [NBLK, 128, Tloc, 8], F32, isOutput=True)

    with tile.TileContext(nc, trace_sim=trace_sim) as tc:
        with (
            tc.tile_pool(name="const", bufs=1) as cpool,
            tc.tile_pool(name="state", bufs=1) as spool,
            tc.tile_pool(name="z", bufs=6) as zpool,
            tc.tile_pool(name="scr", bufs=2) as scr,
            tc.tile_pool(name="ystage", bufs=2) as ypool,
            tc.tile_pool(name="gpsum", bufs=3, space="PSUM") as gp_pool,
            tc.tile_pool(name="hpsum", bufs=2, space="PSUM") as hp_pool,
        ):
            # ---- constants ----
            w_s = cpool.tile([KZ, 136], BF16, name="w_s")
            id_s = cpool.tile([128, 128], BF16, name="id_s")
            nc.sync.dma_start(w_s[:], w_d[:])
            nc.sync.dma_start(id_s[:], id_d[:])
            nc.tensor.ldweights(w_s[:, 0:1])
            nc.tensor.ldweights(id_s[:, 0:1])

            # ---- persistent state: [tg(16) | c(16)] per (blk, cell) ----
            st = spool.tile([128, NBLK * 2 * 32], BF16, name="st")
            stc = spool.tile([128, 1], F32, name="stc")
            nc.vector.memset(st[:], 0.0)
            nc.scalar.copy(stc[:], st[:, 0:1])  # ACT-side carrier for memset sem
            st4 = st.rearrange("p (b c s) -> p b c s", b=NBLK, c=2)

            # ---- z chunk tiles ----
            zs = [zpool.tile([KZ, CHUNK * BC], BF16, name=f"z{k}", tag="z") for k in range(nch)]
            # zero the h rows read before first writes (ticks 0 and 1)
            nc.vector.memset(zs[0][0:32, 0 : 2 * BC], 0.0)
            nc.tensor.ldweights(zs[0][0:32, 0:1])

            def x_dma(k):
                nc.sync.dma_start(zs[k][32:40, :], xq_d[k])
                nc.tensor.ldweights(zs[k][0:40, 0:1])

            for k in range(min(NPRE, nch)):
                x_dma(k)

            ystages = []
            nych = (Tloc + YCH - 1) // YCH
            for k in range(nych):
                ystages.append(
                    ypool.tile([128, YCH * NBLK * 8], F32, name=f"ys{k}", tag="ys")
                )

            # ---- PE warm-up preamble: ~4us of back-to-back matmuls so the
            # HAM clock gate un-throttles the array before the recurrence ----
            wk = gp_pool.tile([128, 1024], F32, name="gp", tag="gp")
            for j in range(16):
                nc.tensor.matmul(wk[0:128, 0:256], id_s[:], st[:])
            del wk

            for t in range(nticks):
                if t % CHUNK == 0 and t // CHUNK + NPRE < nch:
                    x_dma(t // CHUNK + NPRE)

                z = zs[t // CHUNK]
                s = t % CHUNK
                zcol = lambda b: z[0:KZ, s * BC + b * 128 : s * BC + (b + 1) * 128]

                # ---------- matmuls ----------
                # gp spans 2 PSUM banks; blocks at 256-col offsets so each
                # [*, 136] matmul output stays within one bank.
                gp = gp_pool.tile([128, 1024], F32, name="gp", tag="gp")
                gp4 = gp.rearrange("p (b c k) -> p b c k", b=NBLK, c=4)
                nw = 136 if t >= 2 else 128  # y cols only once valid
                for b in range(NBLK):
                    nc.tensor.matmul(
                        gp[:, b * 256 : b * 256 + nw], zcol(b), w_s[:, 0:nw]
                    )

                # ---------- elementwise ----------
                cs = slice(0, 1) if t == 0 else slice(0, 2)

                sg = scr.tile([128, NBLK * 2 * 48], BF16, name="sg", tag="sg")
                sg4 = sg.rearrange("p (b c s) -> p b c s", b=NBLK, c=2)
                prod = scr.tile([128, NBLK * 2 * 32], BF16, name="prod", tag="prod")
                pr4 = prod.rearrange("p (b c s) -> p b c s", b=NBLK, c=2)
                tcs = scr.tile([128, NBLK * 2 * 16], BF16, name="tcs", tag="tcs")
                tc4 = tcs.rearrange("p (b c s) -> p b c s", b=NBLK, c=2)
                hbuf = scr.tile([128, NBLK * 32], BF16, name="hbuf", tag="hbuf")
                hb4 = hbuf.rearrange("p (b c s) -> p b c s", b=NBLK, c=2)

                # sigmoid over [i, f] of active cells (spine)
                nc.scalar.activation(sg4[:, :, cs, 0:32], gp4[:, :, cs, 0:32], AF.Sigmoid)
                # tanh(g) -> st tg slot (spine)
                nc.scalar.activation(st4[:, :, cs, 0:16], gp4[:, :, cs, 48:64], AF.Tanh)
                # [i*tg | f*c]
                nc.vector.tensor_mul(pr4[:, :, cs], sg4[:, :, cs, 0:32], st4[:, :, cs])
                # sigmoid(o) off-spine: overlaps the DVE mul/add
                nc.scalar.activation(sg4[:, :, cs, 32:48], gp4[:, :, cs, 32:48], AF.Sigmoid)
                # c' = i*tg + f*c  (into st c slot)
                nc.vector.tensor_add(
                    st4[:, :, cs, 16:32], pr4[:, :, cs, 0:16], pr4[:, :, cs, 16:32]
                )
                # tanh(c')
                nc.scalar.activation(tc4[:, :, cs], st4[:, :, cs, 16:32], AF.Tanh)
                # h = sig_o * tanh(c')
                nc.vector.tensor_mul(hb4[:, :, cs], sg4[:, :, cs, 32:48], tc4[:, :, cs])

                # ---------- h transpose + single copy ----------
                hT = hp_pool.tile([32, 512], BF16, name="hT", tag="hT")
                ncl = 16 if t == 0 else 32
                for b in range(NBLK):
                    nc.tensor.transpose(
                        hT[0:ncl, b * 128 : (b + 1) * 128],
                        hbuf[:, b * 32 : b * 32 + ncl],
                        id_s[:],
                    )
                zn = zs[(t + 1) // CHUNK]
                sn = (t + 1) % CHUNK
                # two half-copies: blocks 0-1 land first so their next-tick
                # matmuls start while blocks 2-3 are still copying
                nc.vector.tensor_copy(
                    zn[0:ncl, sn * BC : sn * BC + 256], hT[0:ncl, 0:256]
                )
                nc.vector.tensor_copy(
                    zn[0:ncl, sn * BC + 256 : (sn + 1) * BC], hT[0:ncl, 256:512]
                )

                # ---------- y evacuation ----------
                if t >= 2:
                    s3 = t - 2
                    ys = ystages[s3 // YCH]
                    ys4 = ys.rearrange("p (b ts o) -> p b ts o", b=NBLK, ts=YCH)
                    pos = s3 % YCH
                    nc.vector.tensor_copy(
                        ys4[:, :, pos : pos + 1, :], gp4[:, :, 2:3, 0:8]
                    )
                    if pos == YCH - 1 or s3 == Tloc - 1:
                        k = s3 // YCH
                        t0 = k * YCH
                        n = min(YCH, Tloc - t0)
                        src = ystages[k].rearrange(
                            "p (b ts o) -> p b ts o", b=NBLK, ts=YCH
                        )[:, :, 0:n, :]
                        dst = y_d[:, :, t0 : t0 + n, :].rearrange("b p ts o -> p b ts o")
                        nc.sync.dma_start(dst, src)

    return nc


_prog_cache = {}


def _get_program(Tloc):
    if Tloc not in _prog_cache:
        nc = build_program(Tloc)
        nc.finalize()
        _prog_cache[Tloc] = nc
    return _prog_cache[Tloc]


def _prep_weights(W_ih1, W_hh1, b_ih1, b_hh1, W_ih2, W_hh2, b_ih2, b_hh2, W_l, b_l):
    """Build the combined [KZ, 136] weight/bias matrix (f32; cast later)."""
    w = np.zeros((KZ, 136), np.float32)
    # torch gate order rows: [i, f, g, o] x16 ; our col order per cell: [i f o g]
    perm = np.r_[0:16, 16:32, 48:64, 32:48]  # -> [i, f, o, g]

    def cell_cols(c):
        return slice(c * 64, (c + 1) * 64)

    # cell 1 (cols 0:64)
    w[R_H1:R_H1 + 16, 0:64] = W_hh1.T[:, perm]
    w[R_ONE, 0:64] = (b_ih1 + b_hh1)[perm]
    w[R_XT:R_XT + 2, 0:64] = W_ih1.T[:, perm]
    # cell 2 (cols 64:128); x2 = [x, h1]
    w[R_H1:R_H1 + 16, 64:128] = W_ih2.T[2:18][:, perm]
    w[R_H2:R_H2 + 16, 64:128] = W_hh2.T[:, perm]
    w[R_ONE, 64:128] = (b_ih2 + b_hh2)[perm]
    w[R_XT1:R_XT1 + 2, 64:128] = W_ih2.T[0:2][:, perm]
    # y (cols 128:136); x3 = [x, h2]
    w[R_H2:R_H2 + 16, 128:133] = W_l.T[2:18]
    w[R_ONE, 128:133] = b_l
    w[R_XT2:R_XT2 + 2, 128:133] = W_l.T[0:2]
    return w


def _prep_inputs(x, W_ih1, W_hh1, b_ih1, b_hh1, W_ih2, W_hh2, b_ih2, b_hh2, W_l, b_l):
    bf = ml_dtypes.bfloat16
    Tloc = x.shape[1]
    nticks = Tloc + 2
    nch = (nticks + 1 + CHUNK - 1) // CHUNK
    w = _prep_weights(
        W_ih1, W_hh1, b_ih1, b_hh1, W_ih2, W_hh2, b_ih2, b_hh2, W_l, b_l
    ).astype(bf)
    ident = np.eye(128, dtype=np.float32).astype(bf)

    maps = []
    for c in range(NCORES):
        xc = np.ascontiguousarray(x[c * BC : (c + 1) * BC])  # [BC, T, 2]
        xt = np.zeros((nch * CHUNK, 2, BC), np.float32)
        xt[:Tloc] = xc.transpose(1, 2, 0)
        # xq rows: [ones | x(t) | x(t-1) | x(t-2) | zero]
        xq = np.zeros((nch * CHUNK, 8, BC), np.float32)
        xq[:, 0] = 1.0
        xq[:, 1:3] = xt
        xq[1:, 3:5] = xt[:-1]
        xq[2:, 5:7] = xt[:-2]
        xqb = (
            xq.astype(bf)
            .reshape(nch, CHUNK, 8, BC)
            .transpose(0, 2, 1, 3)  # [nch, 8, CHUNK, BC]
            .reshape(nch, 8, CHUNK * BC)
        )
        maps.append(dict(xq=np.ascontiguousarray(xqb), w=w, ident=ident))
    return maps


def _assemble(results, Tloc):
    y = np.empty((B, Tloc, O), np.float32)
    for c in range(NCORES):
        yc = results[c]["y"]  # [NBLK, 128, Tloc, 8]
        y[c * BC : (c + 1) * BC] = yc.reshape(BC, Tloc, 8)[:, :, :O]
    return y


def run(inputs, trace=False, **kw):
    x = np.asarray(inputs["x"])
    Tloc = x.shape[1]
    nc = _get_program(Tloc)
    in_maps = _prep_inputs(**{k: np.asarray(v) for k, v in inputs.items()})
    res = run_bass_kernel_spmd(nc, in_maps, list(range(NCORES)), trace=trace, **kw)
    return _assemble(res.results, Tloc), res


def kernel(**inputs):
    y, _ = run(inputs)
    return y
